# revision 1
# baseline (speedup 1.0000x reference)
"""Trainium2 Bass kernel for nn_MultiHeadAttention_55336358642102.

Strategy: data-parallel over the 8 equal-length sentences (B=8) — one
sentence per NeuronCore, no collectives. Each core computes, for its
[L=1024, D=1024] slice:
  - Q^T/K^T per head via weight-stationary matmuls (heads packed in pairs
    so the PE runs with M=128) on a host-pretransposed X^T; V in natural
    [token, dv] layout directly (lhsT = X^T chunks).
  - attention in "transposed score" space: S^T = K^T-chunks.T @ Q^T so the
    softmaxed probabilities come out with keys on partitions, which is the
    exact layout the P@V matmul needs (lhsT = V-natural chunks).
  - softmax without max-subtraction (logits are ~N(0, 0.15) here — exact
    softmax is shift-invariant so this matches the reference); the
    denominator comes from an all-ones-lhsT matmul over exp(S^T), which
    also replicates it across psum partitions for the normalize step.
  - output projection with the per-head halves packed into two [512, L]
    operands (O1T/O2T) matching w_proj1/w_proj2 row order, then residual +
    unbiased-std layernorm in fp32.

Matmul operands are bf16 (full PE rate); accumulation, residual and
layernorm are fp32. All DRAM inputs are pre-arranged partition-major so
every load is one 2D DMA. Partition-range routing (head halves into
packed operands) is done with SBUF->SBUF DMAs, which unlike the compute
engines can shift partitions.
"""

import sys

import ml_dtypes
import numpy as np

if "/opt/trn_rl_repo" not in sys.path:
    sys.path.insert(0, "/opt/trn_rl_repo")

import concourse.bass as bass
import concourse.mybir as mybir
import concourse.tile as tile
from concourse import bacc
from concourse.bass import ds
from concourse.bass_utils import run_bass_kernel_spmd

P = 128
L = 1024            # rows per core (= max_len; one sentence per core)
DM = 1024           # d_model
DC, DP = 768, 256   # content / positional feature split
NKC, NKP = DC // P, DP // P     # 6, 2 feature chunks
NPAIR = 4
NCORES = 8
INV_TEMPER = 1.0 / 32.0         # 1/sqrt(DM)
EPS = 1e-3
F32 = mybir.dt.float32
BF16 = mybir.dt.bfloat16
AF = mybir.ActivationFunctionType
ALU = mybir.AluOpType
BF16NP = ml_dtypes.bfloat16


def build_nc(apply_ln: bool) -> bass.Bass:
    nc = bacc.Bacc(None, target_bir_lowering=False)

    # all inputs are pre-arranged on the host to be partition-major and
    # contiguous per partition, so every load is a single 2D DMA pattern
    xt = nc.dram_tensor("xt", [P, DM // P, L], BF16, kind="ExternalInput")
    xr = nc.dram_tensor("xr", [L, DM], F32, kind="ExternalInput")
    wc_d = nc.dram_tensor("wc", [P, NPAIR, NKC, 3, P], BF16, kind="ExternalInput")
    wp_d = nc.dram_tensor("wp", [P, NPAIR, NKP, 3, P], BF16, kind="ExternalInput")
    w1_d = nc.dram_tensor("w1", [P, 4, DC], BF16, kind="ExternalInput")
    w2_d = nc.dram_tensor("w2", [P, 4, DP], BF16, kind="ExternalInput")
    if apply_ln:
        lna_d = nc.dram_tensor("lna", [1, DM], F32, kind="ExternalInput")
        lnb_d = nc.dram_tensor("lnb", [1, DM], F32, kind="ExternalInput")
    out_d = nc.dram_tensor("out", [L, DM], F32, kind="ExternalOutput")

    with tile.TileContext(nc) as tc:
        with (
            tc.tile_pool(name="sing", bufs=1) as sing,
            tc.tile_pool(name="wpool", bufs=2) as wpool,
            tc.tile_pool(name="qkt", bufs=2) as qkt,
            tc.tile_pool(name="epool", bufs=4) as epool,
            tc.tile_pool(name="dpool", bufs=4) as dpool,
            tc.tile_pool(name="stg", bufs=6) as stg,
            tc.tile_pool(name="zpool", bufs=2) as zpool,
            tc.tile_pool(name="xpool", bufs=2) as xpool,
            tc.tile_pool(name="stat", bufs=3) as stat,
            tc.tile_pool(name="ps_mm", bufs=3, space="PSUM") as ps_mm,
            tc.tile_pool(name="ps_pv", bufs=3, space="PSUM") as ps_pv,
            tc.tile_pool(name="ps_d", bufs=2, space="PSUM") as ps_d,
        ):
            # ---- resident constants -------------------------------------
            # X^T feature chunks as separate tiles: fine-grained DMA deps so
            # the first QKV matmuls start as soon as their chunk lands
            XTs = []
            for o in range(DM // P):
                xto = sing.tile([P, L], BF16, name=f"xt{o}")
                nc.gpsimd.dma_start(xto, xt[:, o])
                XTs.append(xto)

            ones = sing.tile([P, P], BF16)
            nc.vector.memset(ones, 1.0)

            if apply_ln:
                LNA = sing.tile([1, DM], F32)
                nc.sync.dma_start(LNA, lna_d[:])
                LNB = sing.tile([1, DM], F32)
                nc.sync.dma_start(LNB, lnb_d[:])

            O1T = sing.tile([P, 4, L], BF16)   # packed (head, dv<64) rows x t
            O2T = sing.tile([P, 4, L], BF16)

            lo = slice(0, 64)
            hi = slice(64, 128)

            for j in range(NPAIR):
                # ---- Phase A: QKV for head pair (2j, 2j+1) --------------
                wc = wpool.tile([P, NKC, 3, P], BF16, tag="wc")
                nc.sync.dma_start(wc, wc_d[:, j])
                wp = wpool.tile([P, NKP, 3, P], BF16, tag="wp")
                nc.sync.dma_start(wp, wp_d[:, j])

                # per-head layouts, uniform [content | pos] ordering:
                #   QT/KT [p=dk, head-in-pair, t]
                QT = qkt.tile([P, 2, L], BF16, tag="qt")
                KT = qkt.tile([P, 2, L], BF16, tag="kt")
                V = qkt.tile([P, 8, 2, P], BF16, tag="v")

                for s, DST in ((0, QT), (1, KT)):
                    for half in range(2):
                        hs = ds(half * 512, 512)
                        pc = ps_mm.tile([P, 512], F32, tag="mm")
                        for kc in range(NKC):
                            nc.tensor.matmul(
                                pc, wc[:, kc, s, :], XTs[kc][:, hs],
                                start=(kc == 0), stop=(kc == NKC - 1))
                        pp = ps_mm.tile([P, 512], F32, tag="mm")
                        for kc in range(NKP):
                            nc.tensor.matmul(
                                pp, wp[:, kc, s, :], XTs[NKC + kc][:, hs],
                                start=(kc == 0), stop=(kc == NKP - 1))
                        # shift-free halves go straight from psum to the
                        # packed layout; the other halves stage then DMA
                        # (only DMA can shift partition ranges)
                        nc.any.tensor_copy(DST[lo, 0, hs], pc[lo])
                        nc.any.tensor_copy(DST[hi, 0, hs], pp[hi])
                        sc = stg.tile([P, 512], BF16, tag="sc")
                        nc.any.tensor_copy(sc[hi], pc[hi])
                        sp = stg.tile([P, 512], BF16, tag="sp")
                        nc.any.tensor_copy(sp[lo], pp[lo])
                        nc.gpsimd.dma_start(DST[lo, 1, hs], sc[hi])
                        nc.gpsimd.dma_start(DST[hi, 1, hs], sp[lo])

                # V natural: out[token, dv] = sum_f X^T[f, token] * Wv[f, dv]
                for rc in range(8):
                    rsl = ds(rc * P, P)
                    pv_n = ps_mm.tile([P, 512], F32, tag="mm")
                    for kc in range(NKC):
                        nc.tensor.matmul(
                            pv_n[:, 0:128], XTs[kc][:, rsl], wc[:, kc, 2, :],
                            start=(kc == 0), stop=(kc == NKC - 1))
                    for kc in range(NKP):
                        nc.tensor.matmul(
                            pv_n[:, 128:256], XTs[NKC + kc][:, rsl],
                            wp[:, kc, 2, :],
                            start=(kc == 0), stop=(kc == NKP - 1))
                    # psum cols [h c | h' c | h p | h' p] -> per-head
                    # contiguous [cont|pos] blocks via a strided source AP
                    nc.any.tensor_copy(
                        V[:, rc],
                        pv_n[:, 0:256].rearrange(
                            "p (half head e) -> p head half e",
                            half=2, head=2))

                # ---- Phase B: attention for the two heads ---------------
                for hh in range(2):
                    vb = V[:, :, hh, :]   # [p, chunk, dv]
                    for half in range(2):
                        hs = ds(half * 512, 512)
                        pv = ps_pv.tile([P, 512], F32, tag="pv")
                        dd = ps_d.tile([P, 512], F32, tag="d")
                        es = []
                        for c in range(8):
                            csl = ds(c * P, P)
                            pss = ps_mm.tile([P, 512], F32, tag="mm")
                            nc.tensor.matmul(
                                pss, KT[:, hh, csl],
                                QT[:, hh, hs], start=True, stop=True)
                            e = epool.tile([P, 512], BF16, tag="e")
                            nc.scalar.activation(e, pss, AF.Exp,
                                                 scale=INV_TEMPER)
                            nc.tensor.matmul(
                                pv, vb[:, c], e,
                                start=(c == 0), stop=(c == 7))
                            es.append(e)
                            if c % 2 == 1:   # pairwise level 1 on gpsimd
                                s1 = epool.tile([P, 512], BF16, tag="es")
                                nc.gpsimd.tensor_add(s1, es[c - 1], es[c])
                                es.append(s1)  # positions 8..11 hold sums
                        # levels 2+3 on DVE, then one ones-matmul for the
                        # partition-replicated softmax denominator
                        s5 = epool.tile([P, 512], BF16, tag="es")
                        nc.vector.tensor_add(s5, es[8], es[9])
                        s6 = epool.tile([P, 512], BF16, tag="es")
                        nc.vector.tensor_add(s6, es[10], es[11])
                        s7 = epool.tile([P, 512], BF16, tag="es")
                        nc.vector.tensor_add(s7, s5, s6)
                        nc.tensor.matmul(dd, ones, s7, start=True, stop=True)

                        # normalize by 1/d (already replicated across psum
                        # partitions by the all-ones lhsT), stage, route
                        rd = dpool.tile([P, 512], F32, tag="rd")
                        nc.vector.reciprocal_approx_fast(rd, dd)  # psum->sbuf
                        no = stg.tile([P, 512], BF16, tag="no")
                        nc.vector.tensor_mul(no, pv, rd)
                        # psum rows [o1 | o2] for every head; route to the
                        # packed operands
                        if hh == 0:
                            nc.gpsimd.dma_start(O1T[lo, j, hs], no[lo])
                            nc.gpsimd.dma_start(O2T[lo, j, hs], no[hi])
                        else:
                            nc.gpsimd.dma_start(O1T[hi, j, hs], no[lo])
                            nc.gpsimd.dma_start(O2T[hi, j, hs], no[hi])

            # ---- Phase C: output projection + residual + layernorm ------
            W1 = sing.tile([P, 4, DC], BF16)
            nc.sync.dma_start(W1, w1_d[:])
            W2 = sing.tile([P, 4, DP], BF16)
            nc.sync.dma_start(W2, w2_d[:])
            for t in range(L // P):
                tsl = ds(t * P, P)
                poa = ps_pv.tile([P, 512], F32, tag="pv")   # o1[:, 0:512]
                pob = ps_d.tile([P, 512], F32, tag="d")     # o1[:,512:768] | o2
                for kc in range(4):
                    nc.tensor.matmul(poa, O1T[:, kc, tsl],
                                     W1[:, kc, 0:512],
                                     start=kc == 0, stop=kc == 3)
                for kc in range(4):
                    nc.tensor.matmul(pob[:, 0:256], O1T[:, kc, tsl],
                                     W1[:, kc, 512:768],
                                     start=kc == 0, stop=kc == 3)
                for kc in range(4):
                    nc.tensor.matmul(pob[:, 256:512], O2T[:, kc, tsl],
                                     W2[:, kc, :],
                                     start=kc == 0, stop=kc == 3)

                xts = xpool.tile([P, DM], F32, tag="x")
                nc.sync.dma_start(xts, xr[tsl, :])
                z = zpool.tile([P, DM], F32, tag="z")
                nc.vector.tensor_add(z[:, 0:512], poa, xts[:, 0:512])
                nc.vector.tensor_add(z[:, 512:1024], pob, xts[:, 512:1024])

                stats = stat.tile([P, 2, 6], F32, tag="st")
                nc.vector.bn_stats(stats[:, 0], z[:, 0:512])
                nc.vector.bn_stats(stats[:, 1], z[:, 512:1024])
                mv = stat.tile([P, 2], F32, tag="mv")
                nc.vector.bn_aggr(mv, stats)
                sig = stat.tile([P, 1], F32, tag="sig")
                # unbiased std: sqrt(var * n/(n-1)), then +eps, then 1/x
                nc.scalar.activation(sig, mv[:, 1:2], AF.Sqrt,
                                     scale=float(DM) / (DM - 1))
                nc.vector.tensor_scalar_add(sig, sig, EPS)
                nc.vector.reciprocal_approx_fast(sig, sig)
                nc.vector.tensor_scalar(z, z, mv[:, 0:1], sig,
                                        ALU.subtract, ALU.mult)
                if apply_ln:
                    nc.vector.tensor_mul(z, z, LNA.to_broadcast((P, DM)))
                    nc.vector.tensor_add(z, z, LNB.to_broadcast((P, DM)))
                nc.sync.dma_start(out_d[tsl, :], z)

    nc.finalize()
    return nc


def _part_major(a, p=P):
    """[K*p, ...rest] -> [p, K, ...rest] contiguous (partition-major)."""
    k = a.shape[0] // p
    return np.ascontiguousarray(
        a.reshape((k, p) + a.shape[1:]).swapaxes(0, 1))


def _prep(inp, w_qs1, w_ks1, w_vs1, w_qs2, w_ks2, w_vs2, w_proj1, w_proj2):
    wc = np.empty((NPAIR, DC, 3, P), BF16NP)
    wp = np.empty((NPAIR, DP, 3, P), BF16NP)
    for j in range(NPAIR):
        for s, (wa, wb) in enumerate(((w_qs1, w_qs2), (w_ks1, w_ks2),
                                      (w_vs1, w_vs2))):
            wc[j, :, s, 0:64] = wa[2 * j]
            wc[j, :, s, 64:128] = wa[2 * j + 1]
            if s < 2:   # pos pair swapped for q/k (split-K row groups)
                wp[j, :, s, 0:64] = wb[2 * j + 1]
                wp[j, :, s, 64:128] = wb[2 * j]
            else:       # v keeps natural order
                wp[j, :, s, 0:64] = wb[2 * j]
                wp[j, :, s, 64:128] = wb[2 * j + 1]
    # -> [P, NPAIR, NK, 3, P] partition-major
    wc = np.ascontiguousarray(
        wc.reshape(NPAIR, NKC, P, 3, P).transpose(2, 0, 1, 3, 4))
    wp = np.ascontiguousarray(
        wp.reshape(NPAIR, NKP, P, 3, P).transpose(2, 0, 1, 3, 4))
    w1 = _part_major(np.asarray(w_proj1, np.float32).astype(BF16NP))
    w2 = _part_major(np.asarray(w_proj2, np.float32).astype(BF16NP))

    x = np.ascontiguousarray(np.asarray(inp, np.float32)).reshape(NCORES, L, DM)
    xts = [_part_major(x[b].T.astype(BF16NP)) for b in range(NCORES)]
    return x, xts, wc, wp, w1, w2


_NC_CACHE = {}


def _get_nc(apply_ln):
    if apply_ln not in _NC_CACHE:
        _NC_CACHE[apply_ln] = build_nc(apply_ln)
    return _NC_CACHE[apply_ln]


def kernel(inp, w_qs1, w_ks1, w_vs1, w_qs2, w_ks2, w_vs2, w_proj1, w_proj2,
           ln_a, ln_b, batch_size, max_len, _trace=False):
    inp = np.asarray(inp, np.float32)
    assert int(batch_size) == NCORES and int(max_len) == L
    assert inp.shape == (NCORES * L, DM)

    ln_a = np.asarray(ln_a, np.float32).reshape(-1)
    ln_b = np.asarray(ln_b, np.float32).reshape(-1)
    apply_ln = not (np.all(ln_a == 1.0) and np.all(ln_b == 0.0))

    x, xts, wc, wp, w1, w2 = _prep(
        inp, np.asarray(w_qs1, np.float32), np.asarray(w_ks1, np.float32),
        np.asarray(w_vs1, np.float32), np.asarray(w_qs2, np.float32),
        np.asarray(w_ks2, np.float32), np.asarray(w_vs2, np.float32),
        np.asarray(w_proj1, np.float32), np.asarray(w_proj2, np.float32))

    nc = _get_nc(apply_ln)

    in_maps = []
    for b in range(NCORES):
        m = dict(xt=xts[b], xr=np.ascontiguousarray(x[b]),
                 wc=wc, wp=wp, w1=w1, w2=w2)
        if apply_ln:
            m["lna"] = ln_a.reshape(1, DM)
            m["lnb"] = ln_b.reshape(1, DM)
        in_maps.append(m)

    res = run_bass_kernel_spmd(nc, in_maps, list(range(NCORES)), trace=_trace)
    out = np.concatenate([res.results[b]["out"] for b in range(NCORES)], 0)
    if _trace:
        return out, res
    return out



# revision 2
# speedup vs baseline: 1.3408x; 1.3408x over previous
"""Trainium2 Bass kernel for nn_MultiHeadAttention_55336358642102 (v2).

Data-parallel over the 8 sentences (one per core, no collectives).

Per core, everything runs in fp8-e4m3 DoubleRow matmuls (0.5 cyc/row):
  - Q^T/K^T produced in a [64-partition, 2-subtile] layout: content dims on
    the partition axis, positional dims in the second DR k-subtile.  The
    [cont|pos] concat of the reference becomes a DR contraction, so every
    psum->sbuf copy is partition-shift-free (no SBUF routing DMAs at all).
  - S^T = K^T.T @ Q^T per 128-key chunk as a single 64x2 DR matmul; exp on
    the scalar engine over [128, 2048] psum quads, output fp8 directly.
  - softmax denominator via an accumulating all-ones DR matmul (replicated
    across partitions); 1/D on DVE; the PV psum is normalized and routed
    into the packed projection operands O1T/O2T during its psum drain
    (w_proj rows are host-permuted so no partition shift is needed).
  - output projection in fp8 DR; the residual is added in-psum by a
    4096*identity bf16 matmul over natural-layout bf16 x (so the fp32 x
    never crosses HBM and no vector adds are needed).
  - layernorm: bn_stats/aggr on DVE; sigma = exp(0.5*ln(var)) on the scalar
    engine (same activation table as exp -> no table switch); normalize via
    one tensor_scalar on DVE.

Scales: weights are pre-scaled x64 into fp8, Q/K stored x16, V stored x64,
O stored x64, so the projection psum is 4096*z and layernorm runs on 4096*z
with eps' = 4096*eps (scale-invariant, exact).
"""

import sys

import ml_dtypes
import numpy as np

if "/opt/trn_rl_repo" not in sys.path:
    sys.path.insert(0, "/opt/trn_rl_repo")

import concourse.bass as bass
import concourse.mybir as mybir
import concourse.tile as tile
from concourse import bacc
from concourse.bass import ds
from concourse.bass_utils import run_bass_kernel_spmd

P = 128
L = 1024            # tokens per core (one sentence)
DM = 1024
NCORES = 8
EPS = 1e-3
WS = 64.0           # weight pre-scale into fp8
QS = 16.0           # Q/K storage scale
RES = 4096.0        # psum scale at the layernorm (64*64)
EXP_SCALE = 1.0 / (QS * QS * 32.0)   # psum = 256*S_raw, logits = S_raw/32

F32 = mybir.dt.float32
BF16 = mybir.dt.bfloat16
F8 = mybir.dt.float8e4
AF = mybir.ActivationFunctionType
ALU = mybir.AluOpType
PM = mybir.MatmulPerfMode.DoubleRow
F8NP = ml_dtypes.float8_e4m3
BF16NP = ml_dtypes.bfloat16


def build_nc(apply_ln: bool) -> bass.Bass:
    nc = bacc.Bacc(None, target_bir_lowering=False)

    xt_d = nc.dram_tensor("xt", [P, 8, L], F8, kind="ExternalInput")
    xn_d = nc.dram_tensor("xn", [P, 8, DM], BF16, kind="ExternalInput")
    wqk_d = nc.dram_tensor("wqk", [P, 4, 2, 8, P], F8, kind="ExternalInput")
    wv_d = nc.dram_tensor("wv", [P, 8, 512], F8, kind="ExternalInput")
    w1_d = nc.dram_tensor("w1", [P, 4, 768], F8, kind="ExternalInput")
    w2_d = nc.dram_tensor("w2", [P, 4, 256], F8, kind="ExternalInput")
    id_d = nc.dram_tensor("ident", [P, P], BF16, kind="ExternalInput")
    if apply_ln:
        lna_d = nc.dram_tensor("lna", [P, DM], F32, kind="ExternalInput")
        lnb_d = nc.dram_tensor("lnb", [P, DM], F32, kind="ExternalInput")
    out_d = nc.dram_tensor("out", [L, DM], F32, kind="ExternalOutput")

    with tile.TileContext(nc) as tc:
        with (
            tc.tile_pool(name="sing", bufs=1) as sing,
            tc.tile_pool(name="qkp", bufs=3) as qkp,
            tc.tile_pool(name="ep", bufs=12) as ep,
            tc.tile_pool(name="rdp", bufs=6) as rdp,
            tc.tile_pool(name="outp", bufs=2) as outp,
            tc.tile_pool(name="statp", bufs=2) as statp,
            tc.tile_pool(name="ps", bufs=1, space="PSUM") as ps,
        ):
            # ---- resident inputs -----------------------------------------
            XT = sing.tile([P, 8, L], F8)
            nc.sync.dma_start(XT, xt_d[:])
            WQK = sing.tile([P, 4, 2, 8, P], F8)
            nc.sync.dma_start(WQK, wqk_d[:])
            WV = sing.tile([P, 8, 512], F8)
            nc.sync.dma_start(WV, wv_d[:])
            XN = sing.tile([P, 8, DM], BF16)
            nc.sync.dma_start(XN, xn_d[:])
            W1 = sing.tile([P, 4, 768], F8)
            nc.sync.dma_start(W1, w1_d[:])
            W2 = sing.tile([P, 4, 256], F8)
            nc.sync.dma_start(W2, w2_d[:])
            ID = sing.tile([P, P], BF16)
            nc.sync.dma_start(ID, id_d[:])
            if apply_ln:
                LNA = sing.tile([P, DM], F32)
                nc.sync.dma_start(LNA, lna_d[:])
                LNB = sing.tile([P, DM], F32)
                nc.sync.dma_start(LNB, lnb_d[:])

            ones8 = sing.tile([P, 2, P], F8)
            nc.vector.memset(ones8, 1.0)

            V = sing.tile([P, 8, 1024], F8)       # [key, chunk, head-dv]
            O1T = sing.tile([P, 4, L], F8)        # proj1 stationary rows x t
            O2T = sing.tile([P, 4, L], F8)

            # ---- helpers -------------------------------------------------
            def qk_round(j, s, half):
                """QKV q/k matmuls + drain for (pair j, q-or-k s, token half)."""
                hs = ds(half * 512, 512)
                pq = ps.tile([P, 1024], F32, tag="qk")
                for kk in range(3):      # content: feat chunk pairs (0,1)..(4,5)
                    nc.tensor.matmul(pq[:, 0:512], WQK[:, j, s, 2 * kk:2 * kk + 2, :],
                                     XT[:, 2 * kk:2 * kk + 2, hs],
                                     start=(kk == 0), stop=(kk == 2), perf_mode=PM)
                nc.tensor.matmul(pq[:, 512:1024], WQK[:, j, s, 6:8, :],
                                 XT[:, 6:8, hs], start=True, stop=True, perf_mode=PM)
                dst = (QT[j] if s == 0 else KT[j])
                nc.vector.tensor_scalar(
                    dst[:, :, hs], pq.rearrange("p (s c) -> p s c", s=2),
                    1.0 / 4.0, None, ALU.mult)

            def v_step(rcs):
                """V production for key-chunk list rcs."""
                for rc in rcs:
                    rsl = ds(rc * P, P)
                    pv_ = ps.tile([P, 1024], F32, tag="qk")
                    for kk in range(3):
                        nc.tensor.matmul(pv_[:, 0:512], XT[:, 2 * kk:2 * kk + 2, rsl],
                                         WV[:, 2 * kk:2 * kk + 2, :],
                                         start=(kk == 0), stop=(kk == 2), perf_mode=PM)
                    nc.tensor.matmul(pv_[:, 512:1024], XT[:, 6:8, rsl], WV[:, 6:8, :],
                                     start=True, stop=True, perf_mode=PM)
                    vv = V[:, rc].rearrange("p (j x) -> p j x", j=4)
                    nc.vector.tensor_copy(vv[:, :, 0:64], pv_[:, 0:256].rearrange("p (j e) -> p j e", j=4))
                    nc.vector.tensor_copy(vv[:, :, 192:256], pv_[:, 256:512].rearrange("p (j e) -> p j e", j=4))
                    nc.vector.tensor_copy(vv[:, :, 64:128], pv_[:, 512:768].rearrange("p (j e) -> p j e", j=4))
                    nc.vector.tensor_copy(vv[:, :, 128:192], pv_[:, 768:1024].rearrange("p (j e) -> p j e", j=4))

            # ---- stage 0: pair-0 q/k, then pipelined attention -----------
            QT = [qkp.tile([P, 2, L], F8, tag="qt", name=f"qt{j}") for j in range(4)]
            KT = [qkp.tile([P, 2, L], F8, tag="kt", name=f"kt{j}") for j in range(4)]

            for s in range(2):
                for half in range(2):
                    qk_round(0, s, half)

            v_sched = [[0, 1], [2, 3], [4, 5], [6, 7]]
            qk_sched = [(j, s, half) for j in (1, 2, 3)
                        for s in range(2) for half in range(2)]

            LAG = 4
            pend = []   # (i, pv_psum) awaiting O-drain

            def attn_front(i):
                """S quads + exp + denom + recip for iteration i."""
                j, hh, half = i // 4, (i // 2) % 2, i % 2
                hs = ds(half * 512, 512)
                pb = ds(64 * hh, 64)
                st = ps.tile([P, 2048], F32, tag="s")
                eA = ep.tile([P, 4, 512], F8, tag="e", name=f"eA{i}")
                eB = ep.tile([P, 4, 512], F8, tag="e", name=f"eB{i}")
                for c in range(4):
                    nc.tensor.matmul(st[:, ds(512 * c, 512)],
                                     KT[j][pb, :, ds(128 * c, 128)],
                                     QT[j][pb, :, hs], start=True, stop=True,
                                     perf_mode=PM)
                nc.scalar.activation(eA.rearrange("p a b -> p (a b)"), st,
                                     AF.Exp, scale=EXP_SCALE)
                if i < 4:
                    v_step(v_sched[i])
                for c in range(4):
                    nc.tensor.matmul(st[:, ds(512 * c, 512)],
                                     KT[j][pb, :, ds(128 * (4 + c), 128)],
                                     QT[j][pb, :, hs], start=True, stop=True,
                                     perf_mode=PM)
                nc.scalar.activation(eB.rearrange("p a b -> p (a b)"), st,
                                     AF.Exp, scale=EXP_SCALE)
                if i < len(qk_sched):
                    qk_round(*qk_sched[i])
                dd = ps.tile([P, 512], F32, tag="dd")
                for cc in range(4):
                    e = (eA, eA, eB, eB)[cc]
                    nc.tensor.matmul(dd, ones8, e[:, (cc % 2) * 2:(cc % 2) * 2 + 2, :],
                                     start=(cc == 0), stop=(cc == 3), perf_mode=PM)
                rd = rdp.tile([P, 512], F32, tag="rd", name=f"rd{i}")
                nc.vector.reciprocal_approx_fast(rd, dd)
                return (eA, eB, rd)

            def attn_back(i, eA, eB, rd):
                """PV + normalized O drain for iteration i."""
                j, hh, half = i // 4, (i // 2) % 2, i % 2
                hs = ds(half * 512, 512)
                h = 2 * j + hh
                pv = ps.tile([P, 512], F32, tag="pv")
                for cc in range(4):
                    e = (eA, eA, eB, eB)[cc]
                    nc.tensor.matmul(pv, V[:, 2 * cc:2 * cc + 2, ds(128 * h, 128)],
                                     e[:, (cc % 2) * 2:(cc % 2) * 2 + 2, :],
                                     start=(cc == 0), stop=(cc == 3), perf_mode=PM)
                if hh == 0:
                    nc.vector.tensor_mul(O1T[0:64, j, hs], pv[0:64], rd[0:64])
                    nc.vector.tensor_mul(O2T[64:128, j, hs], pv[64:128], rd[64:128])
                else:
                    nc.vector.tensor_mul(O2T[0:64, j, hs], pv[0:64], rd[0:64])
                    nc.vector.tensor_mul(O1T[64:128, j, hs], pv[64:128], rd[64:128])

            hist = {}
            for i in range(16):
                hist[i] = attn_front(i)
                if i >= LAG:
                    attn_back(i - LAG, *hist.pop(i - LAG))
            for i in range(16 - LAG, 16):
                attn_back(i, *hist.pop(i))

            # ---- phase C: projection + residual + layernorm --------------
            kcor = float(DM) / (DM - 1)
            for tb in range(8):
                tsl = ds(tb * P, P)
                z = ps.tile([P, 1024], F32, tag=("s" if tb % 2 == 0 else "qk"),
                            name=f"z{tb}")
                # residual opens each bank (start zeroes the whole bank), the
                # fp8 projection groups then accumulate on top
                nc.tensor.matmul(z[:, 0:512], ID, XN[:, tb, 0:512],
                                 start=True, stop=True)
                nc.tensor.matmul(z[:, 512:1024], ID, XN[:, tb, 512:1024],
                                 start=True, stop=True)
                for jj in range(2):
                    nc.tensor.matmul(z[:, 0:512], O1T[:, 2 * jj:2 * jj + 2, tsl],
                                     W1[:, 2 * jj:2 * jj + 2, 0:512],
                                     start=False, stop=(jj == 1), perf_mode=PM,
                                     skip_group_check=True)
                for jj in range(2):
                    nc.tensor.matmul(z[:, 512:768], O1T[:, 2 * jj:2 * jj + 2, tsl],
                                     W1[:, 2 * jj:2 * jj + 2, 512:768],
                                     start=False, stop=(jj == 1), perf_mode=PM,
                                     skip_group_check=True)
                for jj in range(2):
                    nc.tensor.matmul(z[:, 768:1024], O2T[:, 2 * jj:2 * jj + 2, tsl],
                                     W2[:, 2 * jj:2 * jj + 2, :],
                                     start=False, stop=(jj == 1), perf_mode=PM,
                                     skip_group_check=True)

                stats = statp.tile([P, 2, 6], F32, tag="st", name=f"st{tb}")
                nc.vector.bn_stats(stats[:, 0], z[:, 0:512])
                nc.vector.bn_stats(stats[:, 1], z[:, 512:1024])
                mv = statp.tile([P, 2], F32, tag="mv", name=f"mv{tb}")
                nc.vector.bn_aggr(mv, stats)
                lnv = statp.tile([P, 1], F32, tag="lnv", name=f"lnv{tb}")
                nc.scalar.activation(lnv, mv[:, 1:2], AF.Ln, scale=kcor)
                sig = statp.tile([P, 1], F32, tag="sig", name=f"sig{tb}")
                nc.scalar.activation(sig, lnv, AF.Exp, scale=0.5)
                nc.vector.tensor_scalar_add(sig, sig, RES * EPS)
                nc.vector.reciprocal_approx_fast(sig, sig)
                ot = outp.tile([P, DM], F32, tag="o", name=f"ot{tb}")
                nc.vector.tensor_scalar(ot, z, mv[:, 0:1], sig,
                                        ALU.subtract, ALU.mult)
                if apply_ln:
                    nc.vector.tensor_mul(ot, ot, LNA)
                    nc.vector.tensor_add(ot, ot, LNB)
                nc.sync.dma_start(out_d[tsl, :], ot)

    nc.finalize()
    return nc


def _prep(inp, w_qs1, w_ks1, w_vs1, w_qs2, w_ks2, w_vs2, w_proj1, w_proj2):
    x = np.ascontiguousarray(np.asarray(inp, np.float32)).reshape(NCORES, L, DM)

    # xt: [P, 8, L] fp8 per core ; xn: [P, 8, DM] bf16 per core
    xts, xns = [], []
    for b in range(NCORES):
        xt = x[b].T.reshape(8, P, L).transpose(1, 0, 2)
        xts.append(np.ascontiguousarray(xt).astype(F8NP))
        xn = x[b].reshape(8, P, DM).transpose(1, 0, 2)
        xns.append(np.ascontiguousarray(xn).astype(BF16NP))

    # wqk: [P, 4, 2, 8, 128] fp8 (x64)
    wqk = np.empty((P, 4, 2, 8, P), np.float32)
    for j in range(4):
        for s, (wa, wb) in enumerate(((w_qs1, w_qs2), (w_ks1, w_ks2))):
            for f in range(6):
                for m_h, h in ((0, 2 * j), (1, 2 * j + 1)):
                    wqk[:, j, s, f, 64 * m_h:64 * m_h + 64] = \
                        wa[h, 128 * f:128 * (f + 1), :]
            for f in (6, 7):
                for m_h, h in ((0, 2 * j), (1, 2 * j + 1)):
                    wqk[:, j, s, f, 64 * m_h:64 * m_h + 64] = \
                        wb[h, 128 * (f - 6):128 * (f - 5), :]
    wqk = (wqk * WS).astype(F8NP)

    # wv: [P, 8, 512] fp8 (x64); head order [0,2,4,6,1,3,5,7]
    horder = [0, 2, 4, 6, 1, 3, 5, 7]
    wv = np.empty((P, 8, 512), np.float32)
    for f in range(6):
        for i, h in enumerate(horder):
            wv[:, f, 64 * i:64 * i + 64] = w_vs1[h, 128 * f:128 * (f + 1), :]
    for f in (6, 7):
        for i, h in enumerate(horder):
            wv[:, f, 64 * i:64 * i + 64] = w_vs2[h, 128 * (f - 6):128 * (f - 5), :]
    wv = (wv * WS).astype(F8NP)

    # w1: rows in natural order [128j + p] ; w2: roll by 64 within 128-blocks
    w1 = np.ascontiguousarray(
        (np.asarray(w_proj1, np.float32) * WS).reshape(4, P, 768)
        .transpose(1, 0, 2)).astype(F8NP)
    w2r = np.asarray(w_proj2, np.float32).reshape(4, 2, 64, 256)
    w2r = np.ascontiguousarray(w2r[:, ::-1].reshape(4, P, 256) * WS)
    w2 = np.ascontiguousarray(w2r.transpose(1, 0, 2)).astype(F8NP)

    ident = (np.eye(P, dtype=np.float32) * RES).astype(BF16NP)
    return x, xts, xns, wqk, wv, w1, w2, ident


_NC_CACHE = {}


def _get_nc(apply_ln):
    if apply_ln not in _NC_CACHE:
        _NC_CACHE[apply_ln] = build_nc(apply_ln)
    return _NC_CACHE[apply_ln]


def kernel(inp, w_qs1, w_ks1, w_vs1, w_qs2, w_ks2, w_vs2, w_proj1, w_proj2,
           ln_a, ln_b, batch_size, max_len, _trace=False):
    inp = np.asarray(inp, np.float32)
    assert int(batch_size) == NCORES and int(max_len) == L
    assert inp.shape == (NCORES * L, DM)

    ln_a = np.asarray(ln_a, np.float32).reshape(-1)
    ln_b = np.asarray(ln_b, np.float32).reshape(-1)
    apply_ln = not (np.all(ln_a == 1.0) and np.all(ln_b == 0.0))

    x, xts, xns, wqk, wv, w1, w2, ident = _prep(
        inp, np.asarray(w_qs1, np.float32), np.asarray(w_ks1, np.float32),
        np.asarray(w_vs1, np.float32), np.asarray(w_qs2, np.float32),
        np.asarray(w_ks2, np.float32), np.asarray(w_vs2, np.float32),
        np.asarray(w_proj1, np.float32), np.asarray(w_proj2, np.float32))

    nc = _get_nc(apply_ln)

    in_maps = []
    for b in range(NCORES):
        m = dict(xt=xts[b], xn=xns[b], wqk=wqk, wv=wv, w1=w1, w2=w2,
                 ident=ident)
        if apply_ln:
            m["lna"] = np.broadcast_to(ln_a, (P, DM)).copy()
            m["lnb"] = np.broadcast_to(ln_b, (P, DM)).copy()
        in_maps.append(m)

    res = run_bass_kernel_spmd(nc, in_maps, list(range(NCORES)), trace=_trace)
    out = np.concatenate([res.results[b]["out"] for b in range(NCORES)], 0)
    if _trace:
        return out, res
    return out
